# revision 10
# baseline (speedup 1.0000x reference)
"""Trainium2 Bass kernel for nn_NodeTreeFunc (gnn_message_passing).

Math per node i (see reference):
    d0_j  = We_ea.T @ relu(ea_j) + We_x.T @ relu(x) + be          j in [16]
    4x tree level (same Ws1/Ws2 each level), pairs (a,b):
        h   = W1a.T relu(d_a) + W1b.T relu(d_b) + W1x.T relu(x) + bs1
        d'  = Ws2[0:128].T relu(h)[0:128] + Ws2[128:256].T relu(h)[128:256] + bs2
    mlp:  a1 = Wm1x.T relu(x) + Wm1s.T relu(d4) + bm1
          out = Wm2.T relu(a1) + bm2 + x

Every intermediate is consumed only through relu, so rectified values are
stored. Compute dtype fp8e4m3 with fp32 PSUM accumulation: all K=256
contractions run as single DoubleRow matmuls (2 fp8 MACs/cell/cycle, the
weight pair stored as a 3D [128,2,128] stationary AP and the moving pair
as adjacent [128,2,T] blocks), which halves TensorE column time vs the
bf16 kernel; K=128 terms (the shared relu(x) chunks of Ws1, and Wm2) run
as plain fp8 matmuls. Matmuls are ordered in same-weight runs so
LDWEIGHTS (not hidden by FWL in DoubleRow mode) amortizes.

Mapping: nodes sharded across 8 cores (data parallel, no collectives).
Feature-major on device ([feature -> partition, node -> free]). The host
does layout only: groups edges by dest (identity for the canonical
input), applies the input relus during the fp8 cast (relu(ea), relu(x) -
monotone, commutes with the cast), pre-pairs relu(ea_j) with relu(x) for
the encode DoubleRow blocks, and packs x + bm2 in fp32 for the final
residual. All weight multiplications, the tree nonlinearities, and the
residual add run on device.

Per core: 10 node tiles of T=512, ~95 matmuls+20 drains per tile; PSUM
as two rotating 4-bank [128,4,512] tiles; two node tiles software-
pipelined (generator interleave); drains (fused bias+relu PSUM->SBUF,
fp8 out) load-balanced between ScalarE and VectorE by a greedy ns-cost
model. Accuracy: l2 rel err ~1e-2 vs the fp32 reference (fp8
quantization noise; gate is 2e-2).
"""

import numpy as np

import concourse.bacc as bacc
import concourse.mybir as mybir
from concourse.bass import ts
from concourse.bass_utils import run_bass_kernel_spmd
from concourse.tile import TileContext

N, D, CH = 40000, 16, 128
NCORES = 8
NC_NODES = N // NCORES      # 5000 nodes per core
T = 500                     # nodes per on-device tile (divides 5000: no padding)
NT = NC_NODES // T
NPAD = NT * T               # == NC_NODES
PS = 512                    # psum slice stride (bank-aligned)

F32 = mybir.dt.float32
FP8 = mybir.dt.float8e4
FP8_NP = mybir.dt.np(mybir.dt.float8e4)

DRMODE = mybir.MatmulPerfMode.DoubleRow

# weight slots inside the packed [128, 13, 128] fp8 weight tensor
W_ENC = 0     # (We[0:128], We[128:256]) DR pair
W_AB0 = 2     # (Ws1[0:128, 0:128], Ws1[128:256, 0:128]) DR pair
W_AB1 = 4     # same, out chunk m=1
W_X0 = 6      # Ws1[256:384, 0:128] plain
W_X1 = 7
W_D = 8       # (Ws2[0:128], Ws2[128:256]) DR pair
W_M1 = 10     # (Wm1[128:256], Wm1[0:128]) DR pair: blocks (summary, x)
W_M2 = 12     # Wm2 plain
# bias columns inside the packed [128, 5] bias tensor
B_E, B_S1A, B_S1B, B_S2, B_M1 = 0, 1, 2, 3, 4

TRACE = False
LAST_RESULT = None

WAVE = 2          # psum slices per wave ([128, WAVE, T] f32 = 2*WAVE banks)
PSUM_BUFS = 4     # WAVE * PSUM_BUFS * 2 banks must be <= 8
INTERLEAVE = 2    # node tiles in flight

# effective drain rates (elems/ns) for the greedy ACT/DVE load balancer
ACT_RATE = 1.2
DVE_RATE = 0.96


def _build_program(iters=1):
    nc = bacc.Bacc()
    ea = nc.declare_dram_parameter("ea", [128, NT * 32 * T], FP8, isOutput=False)
    xrp = nc.declare_dram_parameter("xrp", [128, NPAD], FP8, isOutput=False)
    xbp = nc.declare_dram_parameter("xbp", [128, NPAD], F32, isOutput=False)
    wp = nc.declare_dram_parameter("wp", [128, 13 * 128], FP8, isOutput=False)
    bp = nc.declare_dram_parameter("bp", [128, 5], F32, isOutput=False)
    outT = nc.declare_dram_parameter("outT", [128, NPAD], F32, isOutput=True)

    relu = mybir.ActivationFunctionType.Relu
    add_op = mybir.AluOpType.add
    max_op = mybir.AluOpType.max

    eng_cost = {"act": 0.0, "dve": 0.0}

    with TileContext(nc) as tc:
        with (
            tc.tile_pool(name="consts", bufs=1) as consts,
            tc.tile_pool(name="eap", bufs=2) as ea_pool,
            tc.tile_pool(name="io", bufs=3) as io_pool,
            tc.tile_pool(name="mids", bufs=2) as mids,
            tc.tile_pool(name="psum", bufs=PSUM_BUFS, space="PSUM") as psum_pool,
        ):
            w_sb = consts.tile([128, 13, 128], FP8)
            nc.sync.dma_start(w_sb[:], wp[:, :])
            b_sb = consts.tile([128, 5], F32)
            nc.sync.dma_start(b_sb[:], bp[:, :])

            def bias(col):
                return b_sb[:, col : col + 1]

            def wpair(idx):
                return w_sb[:, idx : idx + 2, :]

            def wone(idx):
                return w_sb[:, idx, :]

            def drain(out_ap, psum_ap, bias_col, fd):
                # fused (psum + bias) -> relu -> fp8 cast, on the cheaper engine
                c_act = (172.0 + fd) / ACT_RATE
                c_dve = (120.0 + fd) / DVE_RATE
                if eng_cost["act"] + c_act <= eng_cost["dve"] + c_dve:
                    eng_cost["act"] += c_act
                    nc.scalar.activation(out_ap, psum_ap, relu, bias=bias(bias_col))
                else:
                    eng_cost["dve"] += c_dve
                    nc.vector.tensor_scalar(
                        out=out_ap,
                        in0=psum_ap,
                        scalar1=bias(bias_col),
                        scalar2=0.0,
                        op0=add_op,
                        op1=max_op,
                    )

            def tile_body(i):
                """Generator: yields after each PSUM wave so two node tiles
                can be software-pipelined against each other."""
                # ---- load node tile ----
                eat = ea_pool.tile([128, 32, T], FP8, tag="eat")
                nc.sync.dma_start(eat[:], ea[:, ts(i, 32 * T)])
                # xt: slot 0 <- relu(d4) drain (written later), slot 1 = relu(x)
                xt = io_pool.tile([128, 2, T], FP8, tag="xt")
                nc.sync.dma_start(xt[:, 1, :], xrp[:, ts(i, T)])
                xbt = io_pool.tile([128, T], F32, tag="xbt")
                nc.sync.dma_start(xbt[:], xbp[:, ts(i, T)])
                yield

                xr = xt[:, 1, :]

                # ---- encode: r0_j = relu(We_ea.T relu(ea_j) + We_x.T xr + be)
                r0 = mids.tile([128, 16, T], FP8, tag="r0")
                for k in range(16 // WAVE):
                    ps = psum_pool.tile([128, WAVE, PS], F32, tag="ps")
                    for j in range(WAVE):
                        s = WAVE * k + j
                        nc.tensor.matmul(
                            ps[:, j, :T], wpair(W_ENC), eat[:, 2 * s : 2 * s + 2, :],
                            start=True, stop=True, perf_mode=DRMODE,
                        )
                    drain(r0[:, WAVE * k : WAVE * (k + 1), :], ps[:, :, :T],
                          B_E, WAVE * T)
                    yield

                # ---- tree levels (shared Ws1/Ws2) ----
                prev = r0
                pairs = 8
                lvl = 0
                while pairs >= 1:
                    lvl += 1
                    # h: [pair-major slots 2*i+m] strided drains per m
                    ht = mids.tile([128, 2 * pairs, T], FP8, tag=f"h{lvl}")
                    for p0 in range(0, pairs, WAVE):
                        np_ = min(WAVE, pairs - p0)
                        for m in range(2):
                            ps = psum_pool.tile([128, WAVE, PS], F32, tag="ps")
                            wab = wpair(W_AB0 if m == 0 else W_AB1)
                            for q in range(np_):
                                nc.tensor.matmul(
                                    ps[:, q, :T], wab,
                                    prev[:, 2 * (p0 + q) : 2 * (p0 + q) + 2, :],
                                    start=True, stop=False, perf_mode=DRMODE,
                                )
                            wx = wone(W_X0 if m == 0 else W_X1)
                            for q in range(np_):
                                nc.tensor.matmul(
                                    ps[:, q, :T], wx, xr, start=False, stop=True,
                                )
                            drain(
                                ht[:, 2 * p0 + m : 2 * (p0 + np_ - 1) + m + 1 : 2, :],
                                ps[:, :np_, :T],
                                B_S1A if m == 0 else B_S1B, np_ * T,
                            )
                            yield
                    # d: r_i = relu(Ws2.T relu(h_i) + bs2)
                    if pairs > 1:
                        dn = mids.tile([128, pairs, T], FP8, tag=f"d{lvl}")
                    for p0 in range(0, pairs, WAVE):
                        np_ = min(WAVE, pairs - p0)
                        ps = psum_pool.tile([128, WAVE, PS], F32, tag="ps")
                        for q in range(np_):
                            nc.tensor.matmul(
                                ps[:, q, :T], wpair(W_D),
                                ht[:, 2 * (p0 + q) : 2 * (p0 + q) + 2, :],
                                start=True, stop=True, perf_mode=DRMODE,
                            )
                        dst = xt[:, 0:1, :] if pairs == 1 else dn[:, p0 : p0 + np_, :]
                        drain(dst, ps[:, :np_, :T], B_S2, np_ * T)
                        yield
                    if pairs > 1:
                        prev = dn
                    pairs //= 2

                # ---- node mlp + residual ----
                ps = psum_pool.tile([128, WAVE, PS], F32, tag="ps")
                nc.tensor.matmul(ps[:, 0, :T], wpair(W_M1), xt[:, 0:2, :],
                                 start=True, stop=True, perf_mode=DRMODE)
                um = mids.tile([128, T], FP8, tag="um")
                drain(um[:], ps[:, 0, :T], B_M1, T)
                yield

                ps2 = psum_pool.tile([128, WAVE, T], F32, tag="ps")
                nc.tensor.matmul(ps2[:, 0, :T], wone(W_M2), um[:],
                                 start=True, stop=True)
                outf = io_pool.tile([128, T], F32, tag="outf")
                nc.vector.tensor_add(outf[:], ps2[:, 0, :T], xbt[:])
                eng_cost["dve"] += (151.0 + T) / DVE_RATE
                nc.sync.dma_start(outT[:, ts(i, T)], outf[:])
                yield

            # drive node tiles interleaved wave-by-wave; a new tile joins
            # only once the youngest active one is STAG waves in, so the
            # serial tree tail of one tile overlaps the dense head of the next
            STAG = 17
            order = [i for _ in range(iters) for i in range(NT)]
            from collections import deque
            pending = deque(order)
            active = deque()
            progress = {}
            while pending or active:
                if len(active) < INTERLEAVE and pending and (
                        not active or progress[id(active[-1])] >= STAG):
                    g = tile_body(pending.popleft())
                    progress[id(g)] = 0
                    active.append(g)
                gen = active.popleft()
                try:
                    next(gen)
                    progress[id(gen)] += 1
                    active.append(gen)
                except StopIteration:
                    del progress[id(gen)]

    nc.finalize()
    return nc


_PROG = None


def _get_prog():
    global _PROG
    if _PROG is None:
        _PROG = _build_program()
    return _PROG


def _prepare_in_maps(x, edge_index, edge_attr, We, be, Ws1, bs1, Ws2, bs2,
                     Wm1, bm1, Wm2, bm2):
    x = np.asarray(x, dtype=np.float32)
    edge_attr = np.asarray(edge_attr, dtype=np.float32)
    assert x.shape == (N, CH) and edge_attr.shape == (N * D, CH)

    # group edges by destination column; identity for the canonical layout
    col = np.asarray(edge_index)[1]
    if not np.array_equal(col, np.repeat(np.arange(N, dtype=col.dtype), D)):
        edge_attr = edge_attr[np.argsort(col, kind="stable")]

    # input relus fold into the fp8 cast (monotone, 0-preserving)
    ea8 = np.maximum(edge_attr, 0.0).astype(FP8_NP)
    xr8 = np.maximum(x, 0.0).astype(FP8_NP)

    We = np.asarray(We, np.float32)
    Ws1 = np.asarray(Ws1, np.float32)
    Ws2 = np.asarray(Ws2, np.float32)
    Wm1 = np.asarray(Wm1, np.float32)
    Wm2 = np.asarray(Wm2, np.float32)
    chunks = [
        We[0:128], We[128:256],                       # W_ENC pair
        Ws1[0:128, 0:128], Ws1[128:256, 0:128],       # W_AB0 pair
        Ws1[0:128, 128:256], Ws1[128:256, 128:256],   # W_AB1 pair
        Ws1[256:384, 0:128], Ws1[256:384, 128:256],   # W_X0, W_X1
        Ws2[0:128], Ws2[128:256],                     # W_D pair
        Wm1[128:256], Wm1[0:128],                     # W_M1 pair (summary, x)
        Wm2,                                          # W_M2
    ]
    wpk = np.stack(chunks, axis=1).astype(FP8_NP)     # [128, 13, 128]
    wpk = np.ascontiguousarray(wpk).reshape(128, 13 * 128)

    bpack = np.zeros((128, 5), np.float32)
    bpack[:, B_E] = np.asarray(be, np.float32)
    bpack[:, B_S1A] = np.asarray(bs1, np.float32)[0:128]
    bpack[:, B_S1B] = np.asarray(bs1, np.float32)[128:256]
    bpack[:, B_S2] = np.asarray(bs2, np.float32)
    bpack[:, B_M1] = np.asarray(bm1, np.float32)

    xb = x + np.asarray(bm2, np.float32)[None, :]

    in_maps = []
    for c in range(NCORES):
        sl = slice(c * NC_NODES, (c + 1) * NC_NODES)
        ea_c = ea8[c * NC_NODES * D : (c + 1) * NC_NODES * D].reshape(NC_NODES, D, CH)
        xr_c = xr8[sl]
        xb_c = xb[sl]

        # encode DR blocks: [ch, tile, slot, (ea|xr), T]
        pairs = np.empty((NT, T, D, 2, CH), FP8_NP)
        pairs[:, :, :, 0, :] = ea_c.reshape(NT, T, D, CH)
        pairs[:, :, :, 1, :] = xr_c.reshape(NT, T, 1, CH)
        ea_t = np.ascontiguousarray(pairs.transpose(4, 0, 2, 3, 1)).reshape(
            128, NT * 32 * T)

        in_maps.append({
            "ea": ea_t,
            "xrp": np.ascontiguousarray(xr_c.T),
            "xbp": np.ascontiguousarray(xb_c.T),
            "wp": wpk,
            "bp": bpack,
        })

    return in_maps


def kernel(**inputs):
    global LAST_RESULT
    in_maps = _prepare_in_maps(**inputs)
    res = run_bass_kernel_spmd(_get_prog(), in_maps, list(range(NCORES)), trace=TRACE)
    LAST_RESULT = res
    outs = [res.results[c]["outT"].T[:NC_NODES] for c in range(NCORES)]
    return np.ascontiguousarray(np.concatenate(outs, axis=0), dtype=np.float32)


# revision 11
# speedup vs baseline: 1.0624x; 1.0624x over previous
"""Trainium2 Bass kernel for nn_NodeTreeFunc (gnn_message_passing).

Math per node i (see reference):
    d0_j  = We_ea.T @ relu(ea_j) + We_x.T @ relu(x) + be          j in [16]
    4x tree level (same Ws1/Ws2 each level), pairs (a,b):
        h   = W1a.T relu(d_a) + W1b.T relu(d_b) + W1x.T relu(x) + bs1
        d'  = Ws2[0:128].T relu(h)[0:128] + Ws2[128:256].T relu(h)[128:256] + bs2
    mlp:  a1 = Wm1x.T relu(x) + Wm1s.T relu(d4) + bm1
          out = Wm2.T relu(a1) + bm2 + x

Every intermediate is consumed only through relu, so rectified values are
stored. Compute dtype fp8e4m3 with fp32 PSUM accumulation: all K=256
contractions run as single DoubleRow matmuls (2 fp8 MACs/cell/cycle, the
weight pair stored as a 3D [128,2,128] stationary AP and the moving pair
as adjacent [128,2,T] blocks), which halves TensorE column time vs the
bf16 kernel; K=128 terms (the shared relu(x) chunks of Ws1, and Wm2) run
as plain fp8 matmuls. Matmuls are ordered in same-weight runs so
LDWEIGHTS (not hidden by FWL in DoubleRow mode) amortizes.

Mapping: nodes sharded across 8 cores (data parallel, no collectives).
Feature-major on device ([feature -> partition, node -> free]). The host
does layout only: groups edges by dest (identity for the canonical
input), applies the input relus during the fp8 cast (relu(ea), relu(x) -
monotone, commutes with the cast), pre-pairs relu(ea_j) with relu(x) for
the encode DoubleRow blocks, and packs x + bm2 in fp32 for the final
residual. All weight multiplications, the tree nonlinearities, and the
residual add run on device.

Per core: 10 node tiles of T=512, ~95 matmuls+20 drains per tile; PSUM
as two rotating 4-bank [128,4,512] tiles; two node tiles software-
pipelined (generator interleave); drains (fused bias+relu PSUM->SBUF,
fp8 out) load-balanced between ScalarE and VectorE by a greedy ns-cost
model. Accuracy: l2 rel err ~1e-2 vs the fp32 reference (fp8
quantization noise; gate is 2e-2).
"""

import numpy as np

import concourse.bacc as bacc
import concourse.mybir as mybir
from concourse.bass import ts
from concourse.bass_utils import run_bass_kernel_spmd
from concourse.tile import TileContext

N, D, CH = 40000, 16, 128
NCORES = 8
NC_NODES = N // NCORES      # 5000 nodes per core
T = 500                     # nodes per on-device tile (divides 5000: no padding)
NT = NC_NODES // T
NPAD = NT * T               # == NC_NODES
PS = 512                    # psum slice stride (bank-aligned)

F32 = mybir.dt.float32
FP8 = mybir.dt.float8e4
FP8_NP = mybir.dt.np(mybir.dt.float8e4)

DRMODE = mybir.MatmulPerfMode.DoubleRow

# weight slots inside the packed [128, 13, 128] fp8 weight tensor
W_ENC = 0     # (We[0:128], We[128:256]) DR pair
W_AB0 = 2     # (Ws1[0:128, 0:128], Ws1[128:256, 0:128]) DR pair
W_AB1 = 4     # same, out chunk m=1
W_X0 = 6      # Ws1[256:384, 0:128] plain
W_X1 = 7
W_D = 8       # (Ws2[0:128], Ws2[128:256]) DR pair
W_M1 = 10     # (Wm1[128:256], Wm1[0:128]) DR pair: blocks (summary, x)
W_M2 = 12     # Wm2 plain
# bias columns inside the packed [128, 5] bias tensor
B_E, B_S1A, B_S1B, B_S2, B_M1 = 0, 1, 2, 3, 4

TRACE = False
LAST_RESULT = None

WAVE = 2          # psum slices per wave ([128, WAVE, T] f32 = 2*WAVE banks)
PSUM_BUFS = 4     # WAVE * PSUM_BUFS * 2 banks must be <= 8
INTERLEAVE = 3    # node tiles in flight

# effective drain rates (elems/ns) for the greedy ACT/DVE load balancer
ACT_RATE = 1.2
DVE_RATE = 0.96


def _build_program(iters=1):
    nc = bacc.Bacc()
    ea = nc.declare_dram_parameter("ea", [128, NT * 32 * T], FP8, isOutput=False)
    xrp = nc.declare_dram_parameter("xrp", [128, NPAD], FP8, isOutput=False)
    xbp = nc.declare_dram_parameter("xbp", [128, NPAD], F32, isOutput=False)
    wp = nc.declare_dram_parameter("wp", [128, 13 * 128], FP8, isOutput=False)
    bp = nc.declare_dram_parameter("bp", [128, 5], F32, isOutput=False)
    outT = nc.declare_dram_parameter("outT", [128, NPAD], F32, isOutput=True)

    relu = mybir.ActivationFunctionType.Relu
    add_op = mybir.AluOpType.add
    max_op = mybir.AluOpType.max

    eng_cost = {"act": 0.0, "dve": 0.0}

    with TileContext(nc) as tc:
        with (
            tc.tile_pool(name="consts", bufs=1) as consts,
            tc.tile_pool(name="eap", bufs=3) as ea_pool,
            tc.tile_pool(name="io", bufs=5) as io_pool,
            tc.tile_pool(name="mids", bufs=3) as mids,
            tc.tile_pool(name="psum", bufs=PSUM_BUFS, space="PSUM") as psum_pool,
        ):
            w_sb = consts.tile([128, 13, 128], FP8)
            nc.sync.dma_start(w_sb[:], wp[:, :])
            b_sb = consts.tile([128, 5], F32)
            nc.sync.dma_start(b_sb[:], bp[:, :])

            def bias(col):
                return b_sb[:, col : col + 1]

            def wpair(idx):
                return w_sb[:, idx : idx + 2, :]

            def wone(idx):
                return w_sb[:, idx, :]

            def drain(out_ap, psum_ap, bias_col, fd):
                # fused (psum + bias) -> relu -> fp8 cast, on the cheaper engine
                c_act = (172.0 + fd) / ACT_RATE
                c_dve = (120.0 + fd) / DVE_RATE
                if eng_cost["act"] + c_act <= eng_cost["dve"] + c_dve:
                    eng_cost["act"] += c_act
                    nc.scalar.activation(out_ap, psum_ap, relu, bias=bias(bias_col))
                else:
                    eng_cost["dve"] += c_dve
                    nc.vector.tensor_scalar(
                        out=out_ap,
                        in0=psum_ap,
                        scalar1=bias(bias_col),
                        scalar2=0.0,
                        op0=add_op,
                        op1=max_op,
                    )

            def tile_body(i):
                """Generator: yields after each PSUM wave so two node tiles
                can be software-pipelined against each other."""
                # ---- load node tile ----
                eat = ea_pool.tile([128, 32, T], FP8, tag="eat")
                nc.sync.dma_start(eat[:], ea[:, ts(i, 32 * T)])
                # xt: slot 0 <- relu(d4) drain (written later), slot 1 = relu(x)
                xt = io_pool.tile([128, 2, T], FP8, tag="xt")
                nc.sync.dma_start(xt[:, 1, :], xrp[:, ts(i, T)])
                xbt = io_pool.tile([128, T], F32, tag="xbt")
                nc.sync.dma_start(xbt[:], xbp[:, ts(i, T)])
                yield

                xr = xt[:, 1, :]

                # ---- encode: r0_j = relu(We_ea.T relu(ea_j) + We_x.T xr + be)
                r0 = mids.tile([128, 16, T], FP8, tag="r0")
                for k in range(16 // WAVE):
                    ps = psum_pool.tile([128, WAVE, PS], F32, tag="ps")
                    for j in range(WAVE):
                        s = WAVE * k + j
                        nc.tensor.matmul(
                            ps[:, j, :T], wpair(W_ENC), eat[:, 2 * s : 2 * s + 2, :],
                            start=True, stop=True, perf_mode=DRMODE,
                        )
                    drain(r0[:, WAVE * k : WAVE * (k + 1), :], ps[:, :, :T],
                          B_E, WAVE * T)
                    yield

                # ---- tree levels (shared Ws1/Ws2) ----
                prev = r0
                pairs = 8
                lvl = 0
                while pairs >= 1:
                    lvl += 1
                    # h: [pair-major slots 2*i+m] strided drains per m
                    ht = mids.tile([128, 2 * pairs, T], FP8, tag=f"h{lvl}")
                    for p0 in range(0, pairs, WAVE):
                        np_ = min(WAVE, pairs - p0)
                        for m in range(2):
                            ps = psum_pool.tile([128, WAVE, PS], F32, tag="ps")
                            wab = wpair(W_AB0 if m == 0 else W_AB1)
                            for q in range(np_):
                                nc.tensor.matmul(
                                    ps[:, q, :T], wab,
                                    prev[:, 2 * (p0 + q) : 2 * (p0 + q) + 2, :],
                                    start=True, stop=False, perf_mode=DRMODE,
                                )
                            wx = wone(W_X0 if m == 0 else W_X1)
                            for q in range(np_):
                                nc.tensor.matmul(
                                    ps[:, q, :T], wx, xr, start=False, stop=True,
                                )
                            drain(
                                ht[:, 2 * p0 + m : 2 * (p0 + np_ - 1) + m + 1 : 2, :],
                                ps[:, :np_, :T],
                                B_S1A if m == 0 else B_S1B, np_ * T,
                            )
                            yield
                    # d: r_i = relu(Ws2.T relu(h_i) + bs2)
                    if pairs > 1:
                        dn = mids.tile([128, pairs, T], FP8, tag=f"d{lvl}")
                    for p0 in range(0, pairs, WAVE):
                        np_ = min(WAVE, pairs - p0)
                        ps = psum_pool.tile([128, WAVE, PS], F32, tag="ps")
                        for q in range(np_):
                            nc.tensor.matmul(
                                ps[:, q, :T], wpair(W_D),
                                ht[:, 2 * (p0 + q) : 2 * (p0 + q) + 2, :],
                                start=True, stop=True, perf_mode=DRMODE,
                            )
                        dst = xt[:, 0:1, :] if pairs == 1 else dn[:, p0 : p0 + np_, :]
                        drain(dst, ps[:, :np_, :T], B_S2, np_ * T)
                        yield
                    if pairs > 1:
                        prev = dn
                    pairs //= 2

                # ---- node mlp + residual ----
                ps = psum_pool.tile([128, WAVE, PS], F32, tag="ps")
                nc.tensor.matmul(ps[:, 0, :T], wpair(W_M1), xt[:, 0:2, :],
                                 start=True, stop=True, perf_mode=DRMODE)
                um = mids.tile([128, T], FP8, tag="um")
                drain(um[:], ps[:, 0, :T], B_M1, T)
                yield

                ps2 = psum_pool.tile([128, WAVE, T], F32, tag="ps")
                nc.tensor.matmul(ps2[:, 0, :T], wone(W_M2), um[:],
                                 start=True, stop=True)
                outf = io_pool.tile([128, T], F32, tag="outf")
                nc.vector.tensor_add(outf[:], ps2[:, 0, :T], xbt[:])
                eng_cost["dve"] += (151.0 + T) / DVE_RATE
                nc.sync.dma_start(outT[:, ts(i, T)], outf[:])
                yield

            # drive node tiles interleaved wave-by-wave; a new tile joins
            # only once the youngest active one is STAG waves in, so the
            # serial tree tail of one tile overlaps the dense head of the next
            STAG = 12
            order = [i for _ in range(iters) for i in range(NT)]
            from collections import deque
            pending = deque(order)
            active = deque()
            progress = {}
            while pending or active:
                if len(active) < INTERLEAVE and pending and (
                        not active or progress[id(active[-1])] >= STAG):
                    g = tile_body(pending.popleft())
                    progress[id(g)] = 0
                    active.append(g)
                gen = active.popleft()
                try:
                    next(gen)
                    progress[id(gen)] += 1
                    active.append(gen)
                except StopIteration:
                    del progress[id(gen)]

    nc.finalize()
    return nc


_PROG = None


def _get_prog():
    global _PROG
    if _PROG is None:
        _PROG = _build_program()
    return _PROG


def _prepare_in_maps(x, edge_index, edge_attr, We, be, Ws1, bs1, Ws2, bs2,
                     Wm1, bm1, Wm2, bm2):
    x = np.asarray(x, dtype=np.float32)
    edge_attr = np.asarray(edge_attr, dtype=np.float32)
    assert x.shape == (N, CH) and edge_attr.shape == (N * D, CH)

    # group edges by destination column; identity for the canonical layout
    col = np.asarray(edge_index)[1]
    if not np.array_equal(col, np.repeat(np.arange(N, dtype=col.dtype), D)):
        edge_attr = edge_attr[np.argsort(col, kind="stable")]

    # input relus fold into the fp8 cast (monotone, 0-preserving)
    ea8 = np.maximum(edge_attr, 0.0).astype(FP8_NP)
    xr8 = np.maximum(x, 0.0).astype(FP8_NP)

    We = np.asarray(We, np.float32)
    Ws1 = np.asarray(Ws1, np.float32)
    Ws2 = np.asarray(Ws2, np.float32)
    Wm1 = np.asarray(Wm1, np.float32)
    Wm2 = np.asarray(Wm2, np.float32)
    chunks = [
        We[0:128], We[128:256],                       # W_ENC pair
        Ws1[0:128, 0:128], Ws1[128:256, 0:128],       # W_AB0 pair
        Ws1[0:128, 128:256], Ws1[128:256, 128:256],   # W_AB1 pair
        Ws1[256:384, 0:128], Ws1[256:384, 128:256],   # W_X0, W_X1
        Ws2[0:128], Ws2[128:256],                     # W_D pair
        Wm1[128:256], Wm1[0:128],                     # W_M1 pair (summary, x)
        Wm2,                                          # W_M2
    ]
    wpk = np.stack(chunks, axis=1).astype(FP8_NP)     # [128, 13, 128]
    wpk = np.ascontiguousarray(wpk).reshape(128, 13 * 128)

    bpack = np.zeros((128, 5), np.float32)
    bpack[:, B_E] = np.asarray(be, np.float32)
    bpack[:, B_S1A] = np.asarray(bs1, np.float32)[0:128]
    bpack[:, B_S1B] = np.asarray(bs1, np.float32)[128:256]
    bpack[:, B_S2] = np.asarray(bs2, np.float32)
    bpack[:, B_M1] = np.asarray(bm1, np.float32)

    xb = x + np.asarray(bm2, np.float32)[None, :]

    in_maps = []
    for c in range(NCORES):
        sl = slice(c * NC_NODES, (c + 1) * NC_NODES)
        ea_c = ea8[c * NC_NODES * D : (c + 1) * NC_NODES * D].reshape(NC_NODES, D, CH)
        xr_c = xr8[sl]
        xb_c = xb[sl]

        # encode DR blocks: [ch, tile, slot, (ea|xr), T]
        pairs = np.empty((NT, T, D, 2, CH), FP8_NP)
        pairs[:, :, :, 0, :] = ea_c.reshape(NT, T, D, CH)
        pairs[:, :, :, 1, :] = xr_c.reshape(NT, T, 1, CH)
        ea_t = np.ascontiguousarray(pairs.transpose(4, 0, 2, 3, 1)).reshape(
            128, NT * 32 * T)

        in_maps.append({
            "ea": ea_t,
            "xrp": np.ascontiguousarray(xr_c.T),
            "xbp": np.ascontiguousarray(xb_c.T),
            "wp": wpk,
            "bp": bpack,
        })

    return in_maps


def kernel(**inputs):
    global LAST_RESULT
    in_maps = _prepare_in_maps(**inputs)
    res = run_bass_kernel_spmd(_get_prog(), in_maps, list(range(NCORES)), trace=TRACE)
    LAST_RESULT = res
    outs = [res.results[c]["outT"].T[:NC_NODES] for c in range(NCORES)]
    return np.ascontiguousarray(np.concatenate(outs, axis=0), dtype=np.float32)
